# revision 1
# baseline (speedup 1.0000x reference)
# Causal self-attention (B=8, T=1024, C=1024, H=16, D=64) on 8 trn2 NeuronCores.
# Sharding: data-parallel over batch — core i computes batch element i entirely
# (weights replicated, no collectives).
#
# Per-core pipeline (all matmuls bf16 inputs, fp32 PSUM accumulation):
#   0. x [T,C] --cast-dma--> bf16, PE-transpose -> xT [C,T] (8 chunks of [128, T])
#   1. v[t,j] = lhsT=xT chunk, rhs=W_v slices (+bias ones-row mm), stored per-head
#      with a ones column (width 65) so attn@v also yields the softmax denominator.
#   2. interleaved per head-pair hp: qkT[j,t] for jt=hp and 8+hp (lhsT=W column
#      slice, rhs=xT), then attention for heads 2hp, 2hp+1:
#      per (head, 512-query tile): scores sT[j,i] (K=64, causally trimmed),
#      diag-block mask add (DVE), exp on ACT (scale=1/8) -> pT bf16,
#      attn@v (M=65; row 64 = l = sum_j p), reciprocal_approx from PSUM,
#      K=1 matmul broadcast, ACT copy, DVE mult -> oT [c_in, t] bf16.
#   3. out[t,c] : lhsT=oT chunk, rhs=W_out (+bias ones-row mm) -> DMA to DRAM.
#
# W_qkv is DMA'd in column slices (per jt / per v-half) so matmuls start as
# soon as their slice lands; x is DMA'd first.

import numpy as np
from contextlib import ExitStack

import concourse.bass as bass
import concourse.bacc as bacc
import concourse.mybir as mybir
import concourse.tile as tile
from concourse import bass_utils
from concourse.masks import make_identity

FP32 = mybir.dt.float32
BF16 = mybir.dt.bfloat16

B, T, C = 8, 1024, 1024
H, D = 16, 64
N_CORES = 8
MASK_VAL = -1e4  # pre-scale additive mask; exp(0.125 * (s + MASK_VAL)) == 0.0
CCH = C // 128   # 8 contraction chunks of 128
TCH = T // 128   # 8 token chunks of 128


def build_nc():
    nc = bacc.Bacc("TRN2", debug=False, num_devices=N_CORES)

    x_d = nc.dram_tensor("x_b", [T, C], FP32, kind="ExternalInput").ap()
    wq_d = nc.dram_tensor("w_qkv", [C, 3 * C], FP32, kind="ExternalInput").ap()
    bq_d = nc.dram_tensor("b_qkv", [1, 3 * C], FP32, kind="ExternalInput").ap()
    wo_d = nc.dram_tensor("w_out", [C, C], FP32, kind="ExternalInput").ap()
    bo_d = nc.dram_tensor("b_out", [1, C], FP32, kind="ExternalInput").ap()
    out_d = nc.dram_tensor("out_b", [T, C], FP32, kind="ExternalOutput").ap()

    with tile.TileContext(nc) as tc, ExitStack() as ctx:
        consts = ctx.enter_context(tc.tile_pool(name="consts", bufs=1))
        wpool = ctx.enter_context(tc.tile_pool(name="weights", bufs=1))
        apool = ctx.enter_context(tc.tile_pool(name="acts", bufs=1))
        ppool = ctx.enter_context(tc.tile_pool(name="ppool", bufs=8))
        lpool = ctx.enter_context(tc.tile_pool(name="lpool", bufs=2))
        rbpool = ctx.enter_context(tc.tile_pool(name="rbpool", bufs=2))
        outs = ctx.enter_context(tc.tile_pool(name="outs", bufs=2))
        # PSUM: P1 generic [128,512] (projections), P2 scores+bcast, P3 attn out
        P1 = ctx.enter_context(tc.tile_pool(name="P1", bufs=2, space="PSUM"))
        P2 = ctx.enter_context(tc.tile_pool(name="P2", bufs=2, space="PSUM"))
        P3 = ctx.enter_context(tc.tile_pool(name="P3", bufs=2, space="PSUM"))

        # ---- constants (tiny DMAs first) ----
        bqkv_sb = consts.tile([1, 3 * C], BF16, tag="bqkv")
        nc.gpsimd.dma_start(out=bqkv_sb, in_=bq_d)
        bout_sb = consts.tile([1, C], BF16, tag="bout")
        nc.gpsimd.dma_start(out=bout_sb, in_=bo_d)

        identity = consts.tile([128, 128], BF16, tag="identity")
        make_identity(nc, identity)
        # diag_mask[jj, ii] = 0 if ii >= jj else MASK_VAL  (valid = key <= query)
        diag_mask = consts.tile([128, 128], FP32, tag="diag_mask")
        nc.gpsimd.memset(diag_mask, 0.0)
        nc.gpsimd.affine_select(
            out=diag_mask, in_=diag_mask,
            compare_op=mybir.AluOpType.is_ge, fill=MASK_VAL,
            base=0, channel_multiplier=-1, pattern=[[1, 128]],
        )
        ones_row = consts.tile([1, T], BF16, tag="ones_row")
        nc.vector.memset(ones_row, 1.0)
        ones64 = consts.tile([1, 64], FP32, tag="ones64")
        nc.vector.memset(ones64, 1.0)
        # b_qkv for the q/k part transposed to per-partition layout [128, 16]
        bqT = consts.tile([128, 16], FP32, tag="bqT")
        nc.gpsimd.dma_start(
            out=bqT,
            in_=bq_d[:, 0:2 * C].rearrange("x (jt p) -> p (x jt)", p=128))

        # ---- persistent activations ----
        xT = [apool.tile([128, T], BF16, tag=f"xT{cc}", name=f"xT{cc}")
              for cc in range(CCH)]
        qkT = [apool.tile([128, T], BF16, tag=f"qkT{jt}", name=f"qkT{jt}")
               for jt in range(16)]
        vp = [apool.tile([128, H * (D + 1)], BF16, tag=f"vp{t_}", name=f"vp{t_}")
              for t_ in range(TCH)]
        oT = [apool.tile([128, T], BF16, tag=f"oT{hc}", name=f"oT{hc}")
              for hc in range(CCH)]

        # ---- phase 0: x load (cast bf16, first in DMA queue) + PE transpose ----
        with tc.tile_pool(name="xstage", bufs=4) as xstage:
            xs_all = []
            for ti in range(TCH):
                xs = xstage.tile([128, C], BF16, tag="xs", name=f"xs{ti}")
                nc.gpsimd.dma_start(out=xs, in_=x_d[ti * 128:(ti + 1) * 128, :])
                xs_all.append(xs)

            # ---- weight DMAs (column-sliced so consumers start early) ----
            # v part: W_qkv[:, 2C + jvt*512 : ...] as [p, cc, j]
            wv_col = []
            for jvt in range(2):
                t_ = wpool.tile([128, CCH, 512], BF16, tag=f"wv{jvt}", name=f"wv{jvt}")
                src = wq_d[:, 2 * C + jvt * 512: 2 * C + (jvt + 1) * 512]
                nc.gpsimd.dma_start(
                    out=t_, in_=src.rearrange("(cc p) j -> p cc j", p=128))
                wv_col.append(t_)
            # q/k parts per jt, in head-pair consumption order
            wq_col = [None] * 16
            for hp in range(8):
                for jt in (hp, 8 + hp):
                    t_ = wpool.tile([128, CCH, 128], BF16,
                                    tag=f"wq{jt}", name=f"wq{jt}")
                    src = wq_d[:, jt * 128:(jt + 1) * 128]
                    nc.gpsimd.dma_start(
                        out=t_, in_=src.rearrange("(cc p) j -> p cc j", p=128))
                    wq_col[jt] = t_
            # out projection weights, one DMA (needed last)
            wo_col = wpool.tile([128, CCH, C], BF16, tag="wo", name="wo")
            nc.gpsimd.dma_start(
                out=wo_col, in_=wo_d.rearrange("(cc p) j -> p cc j", p=128))

            # transposes: x[t,c] 128x128 blocks -> xT[c,t]
            for ti in range(TCH):
                for cc in range(CCH):
                    pt = P3.tile([128, 128], BF16, tag="po", name="tp")
                    nc.tensor.transpose(
                        out=pt, in_=xs_all[ti][:, cc * 128:(cc + 1) * 128],
                        identity=identity)
                    nc.vector.tensor_copy(
                        out=xT[cc][:, ti * 128:(ti + 1) * 128], in_=pt)

            # broadcast biases across partitions once (K=1 matmul + copy):
            # kills the per-tile ones-row bias matmuls in v/out projections.
            bvb = consts.tile([128, C], FP32, tag="bvb")
            bob = consts.tile([128, C], FP32, tag="bob")
            for half in range(2):
                sl = slice(half * 512, (half + 1) * 512)
                pb = P1.tile([128, 512], FP32, tag="p1", name="pbias")
                nc.tensor.matmul(
                    out=pb, lhsT=ones_row[0:1, 0:128],
                    rhs=bqkv_sb[0:1, 2 * C + half * 512:2 * C + (half + 1) * 512],
                    start=True, stop=True)
                nc.vector.tensor_copy(out=bvb[:, sl], in_=pb)
                pb2 = P1.tile([128, 512], FP32, tag="p1", name="pbias2")
                nc.tensor.matmul(out=pb2, lhsT=ones_row[0:1, 0:128],
                                 rhs=bout_sb[0:1, sl], start=True, stop=True)
                nc.vector.tensor_copy(out=bob[:, sl], in_=pb2)

            # ---- v projection (natural layout, into vp with stride 65) ----
            for ti in range(TCH):
                vcol = vp[ti].rearrange("p (h d) -> p h d", h=H)
                nc.vector.memset(vcol[:, :, D:D + 1], 1.0)
                for jvt in range(2):
                    ps = P1.tile([128, 512], FP32, tag="p1", name="psv")
                    for cc in range(CCH):
                        nc.tensor.matmul(
                            out=ps,
                            lhsT=xT[cc][:, ti * 128:(ti + 1) * 128],
                            rhs=wv_col[jvt][:, cc, :],
                            start=(cc == 0), stop=(cc == CCH - 1))
                    nc.vector.tensor_tensor(
                        out=vcol[:, jvt * 8:(jvt + 1) * 8, 0:D],
                        in0=ps.rearrange("p (h d) -> p h d", h=8),
                        in1=bvb[:, jvt * 512:(jvt + 1) * 512].rearrange(
                            "p (h d) -> p h d", h=8),
                        op=mybir.AluOpType.add)

            # ---- interleaved: qk projection + attention, software-pipelined
            # (qk groups of pair hp+1 are emitted between the attention units
            # of pair hp so PE always has dense independent work queued) ----
            def qk_group(jt, half):
                sl = slice(half * 512, (half + 1) * 512)
                ps = P1.tile([128, 512], FP32, tag="p1", name="psqk")
                for cc in range(CCH):
                    nc.tensor.matmul(
                        out=ps,
                        lhsT=wq_col[jt][:, cc, :],
                        rhs=xT[cc][:, sl],
                        start=(cc == 0), stop=(cc == CCH - 1))
                # bias folded into the copy (per-partition scalar)
                nc.vector.tensor_scalar_add(
                    out=qkT[jt][:, sl], in0=ps, scalar1=bqT[:, jt:jt + 1])

            qk_queue = [(jt, half) for hp in range(8)
                        for jt in (hp, 8 + hp) for half in range(2)]
            for g in qk_queue[:4]:
                qk_group(*g)
            qk_pos = 4

            for hp in range(8):
                # attention for the head pair, jointly: both heads' score
                # matmuls are adjacent (K=64 at partition bases 0/64 -> PE
                # row-groups can overlap) and share one batched exp.
                h0, h1 = 2 * hp, 2 * hp + 1
                qk_q, qk_k = qkT[hp], qkT[8 + hp]
                for it in range(2):
                    njc = 4 * (it + 1)
                    po2 = [P3.tile([65, 512], FP32, tag="po", name=f"po{hx}")
                           for hx in range(2)]
                    for jc in range(njc):
                        s0 = max(0, jc * 128 - it * 512)
                        ps = P2.tile([128, 2, 512], FP32, tag="ps", name="pss")
                        for hx, h in enumerate((h0, h1)):
                            prow = slice(hx * 64, hx * 64 + 64)
                            nc.tensor.matmul(
                                out=ps[:, hx, s0:512],
                                lhsT=qk_k[prow, jc * 128:(jc + 1) * 128],
                                rhs=qk_q[prow, it * 512 + s0:(it + 1) * 512],
                                start=True, stop=True)
                        if jc >= it * 4:  # diagonal block cols [s0, s0+128)
                            for hx in range(2):
                                nc.vector.tensor_tensor(
                                    out=ps[:, hx, s0:s0 + 128],
                                    in0=ps[:, hx, s0:s0 + 128],
                                    in1=diag_mask, op=mybir.AluOpType.add)
                        pT = ppool.tile([128, 2, 512], BF16, tag="pT", name="pT")
                        nc.scalar.activation(
                            out=pT[:, :, s0:512], in_=ps[:, :, s0:512],
                            func=mybir.ActivationFunctionType.Exp, scale=0.125)
                        # PE filler while ACT computes the exp: one qk group
                        if jc % 2 == 1 and qk_pos < len(qk_queue):
                            qk_group(*qk_queue[qk_pos])
                            qk_pos += 1
                        for hx, h in enumerate((h0, h1)):
                            hsl = slice(h * (D + 1), h * (D + 1) + D + 1)
                            nc.tensor.matmul(
                                out=po2[hx][0:65, s0:512],
                                lhsT=vp[jc][:, hsl],
                                rhs=pT[:, hx, s0:512],
                                start=(jc == 0), stop=(jc == njc - 1),
                                skip_group_check=True)
                    # normalize: row 64 of po = l = sum_j p
                    for hx in range(2):
                        po = po2[hx]
                        prow = slice(hx * 64, hx * 64 + 64)
                        l_sb = lpool.tile([1, 512], FP32, tag="l", name="l")
                        nc.scalar.copy(out=l_sb, in_=po[64:65, :])
                        plb = P2.tile([64, 512], FP32, tag="ps", name="plb")
                        nc.tensor.matmul(out=plb, lhsT=ones64, rhs=l_sb,
                                         start=True, stop=True)
                        rb = rbpool.tile([64, 512], FP32, tag="rb", name="rb")
                        nc.vector.reciprocal_approx_fast(out=rb, in_=plb)
                        nc.vector.tensor_tensor(
                            out=oT[hp][prow, it * 512:(it + 1) * 512],
                            in0=po[0:64, :], in1=rb, op=mybir.AluOpType.mult)

            # ---- output projection ----
            for ti in range(TCH):
                ot = outs.tile([128, C], FP32, tag="ot", name="ot")
                for half in range(2):
                    sl = slice(half * 512, (half + 1) * 512)
                    ps = P1.tile([128, 512], FP32, tag="p1", name="pso")
                    for hc in range(CCH):
                        nc.tensor.matmul(
                            out=ps,
                            lhsT=oT[hc][:, ti * 128:(ti + 1) * 128],
                            rhs=wo_col[:, hc, sl],
                            start=(hc == 0), stop=(hc == CCH - 1))
                    nc.vector.tensor_tensor(
                        out=ot[:, sl], in0=ps, in1=bob[:, sl],
                        op=mybir.AluOpType.add)
                nc.sync.dma_start(out=out_d[ti * 128:(ti + 1) * 128, :], in_=ot)

    nc.compile()
    nc.finalize()
    return nc


_CACHE = {}


def kernel(x, W_qkv, b_qkv, W_out, b_out):
    if "nc" not in _CACHE:
        _CACHE["nc"] = build_nc()
    nc = _CACHE["nc"]
    x = np.ascontiguousarray(np.asarray(x, dtype=np.float32))
    in_maps = [
        {
            "x_b": x[i],
            "w_qkv": np.ascontiguousarray(np.asarray(W_qkv, np.float32)),
            "b_qkv": np.ascontiguousarray(np.asarray(b_qkv, np.float32).reshape(1, -1)),
            "w_out": np.ascontiguousarray(np.asarray(W_out, np.float32)),
            "b_out": np.ascontiguousarray(np.asarray(b_out, np.float32).reshape(1, -1)),
        }
        for i in range(N_CORES)
    ]
    res = bass_utils.run_bass_kernel_spmd(nc, in_maps, core_ids=list(range(N_CORES)))
    return np.stack([r["out_b"] for r in res.results]).astype(np.float32)



# revision 3
# speedup vs baseline: 1.1256x; 1.1256x over previous
# Causal self-attention (B=8, T=1024, C=1024, H=16, D=64) on 8 trn2 NeuronCores.
# Sharding: data-parallel over batch — core i computes batch element i entirely
# (weights replicated, no collectives).
#
# Per-core pipeline (matmuls bf16 inputs, fp32 PSUM accumulation):
#   0. x cast-loaded bf16 in 4 chunks; PE-transpose to xT [c, t] pipelined
#      behind the DMA; dummy warm-up matmuls keep the PE HAM clock at 8/8.
#   1. v proj per (ti, jvt): lhsT=xT chunk, rhs=W_v slice -> vp (with ones col
#      at D so attn@v also yields the softmax denominator l). No v-bias: it is
#      folded into the output bias (b~ = b_v @ W_out + b_out, computed on PE).
#   2. qk proj per (jt, half): lhsT=W column slice, rhs=xT -> qkT[j, t].
#      k-bias dropped entirely (softmax is shift-invariant per query);
#      q-bias added by a K=1 matmul (bq x ones row) inside the accumulation.
#   3. attention per head-pair hp, 512-query tile it: scores sT[j,i] (K=64,
#      two heads at partition bases 0/64 run concurrently), exp on ACT
#      (scale=1/8) -> pT bf16, diag-block causal fix = bf16 multiply by a 0/1
#      mask, attn@v (M=65; row 64 = l), l -> reciprocal -> oT [c_in, t].
#      Remaining qk groups / v-proj-jvt1 / bias-fold work are pulled from a
#      queue between attention steps so the PE never idles.
#   4. out proj per ti: lhsT=oT chunk, rhs=W_out, bias via K=1 matmul
#      (ones x b~ row) -> DMA out.
#
# DMA: weights loaded in wide column blocks (2-4KB descriptor lines) in exact
# consumption order; gpsimd Q7 does nothing but descriptor generation.

import numpy as np
import ml_dtypes
from contextlib import ExitStack

import concourse.bass as bass
import concourse.bacc as bacc
import concourse.mybir as mybir
import concourse.tile as tile
from concourse import bass_utils

FP32 = mybir.dt.float32
BF16 = mybir.dt.bfloat16

B, T, C = 8, 1024, 1024
H, D = 16, 64
N_CORES = 8
CCH = C // 128   # 8 contraction chunks of 128
TCH = T // 128   # 8 token chunks of 128


def build_nc():
    nc = bacc.Bacc("TRN2", debug=False, num_devices=N_CORES)

    x_d = nc.dram_tensor("x_b", [T, C], FP32, kind="ExternalInput").ap()
    wq_d = nc.dram_tensor("w_qkv", [C, 3 * C], FP32, kind="ExternalInput").ap()
    bq_d = nc.dram_tensor("b_qkv", [1, 3 * C], FP32, kind="ExternalInput").ap()
    wo_d = nc.dram_tensor("w_out", [C, C], FP32, kind="ExternalInput").ap()
    bo_d = nc.dram_tensor("b_out", [1, C], FP32, kind="ExternalInput").ap()
    id_d = nc.dram_tensor("ident", [128, 128], BF16, kind="ExternalInput").ap()
    mk_d = nc.dram_tensor("mask01", [128, 128], BF16, kind="ExternalInput").ap()
    out_d = nc.dram_tensor("out_b", [T, C], FP32, kind="ExternalOutput").ap()

    with tile.TileContext(nc) as tc, ExitStack() as ctx:
        consts = ctx.enter_context(tc.tile_pool(name="consts", bufs=1))
        wpool = ctx.enter_context(tc.tile_pool(name="weights", bufs=1))
        apool = ctx.enter_context(tc.tile_pool(name="acts", bufs=1))
        xpool = ctx.enter_context(tc.tile_pool(name="xstage", bufs=2))
        ppool = ctx.enter_context(tc.tile_pool(name="ppool", bufs=4))
        lpool = ctx.enter_context(tc.tile_pool(name="lpool", bufs=2))
        rbpool = ctx.enter_context(tc.tile_pool(name="rbpool", bufs=2))
        outs = ctx.enter_context(tc.tile_pool(name="outs", bufs=2))
        # PSUM: one pool, 8 banks exactly:
        #   tag "ps"  [128,2,512] f32 x2  = 4 banks (scores; also vproj/qk
        #             openers, plb, out-proj accums — all <= the slot size)
        #   tag "po"  [65,512]    f32 x3  = 3 banks (attn out + transposes)
        #   tag "aux" [128,512]   f32 x1  = 1 bank  (in-attention fillers)
        PS = ctx.enter_context(tc.tile_pool(name="PS", bufs=2, space="PSUM"))

        # ---- host constants on the HWDGE queue (no Q7 time) ----
        ident = consts.tile([128, 128], BF16, tag="ident", name="ident")
        nc.sync.dma_start(out=ident, in_=id_d)
        mask01 = consts.tile([128, 128], BF16, tag="mask01", name="mask01")
        nc.sync.dma_start(out=mask01, in_=mk_d)

        # ---- vector-engine constants ----
        ones_row = consts.tile([1, 512], BF16, tag="ones_row", name="ones_row")
        nc.vector.memset(ones_row, 1.0)
        ones64 = consts.tile([1, 64], BF16, tag="ones64", name="ones64")
        nc.vector.memset(ones64, 1.0)
        scratch = consts.tile([128, 512], BF16, tag="scratch", name="scratch")
        nc.vector.memset(scratch, 1.0)

        # ---- gpsimd cast-DMAs, exact consumption order ----
        bqkv_sb = consts.tile([1, 3 * C], BF16, tag="bqkv", name="bqkv_sb")
        nc.gpsimd.dma_start(out=bqkv_sb, in_=bq_d)
        bout_sb = consts.tile([1, C], BF16, tag="bout", name="bout_sb")
        nc.gpsimd.dma_start(out=bout_sb, in_=bo_d)

        xs = []
        for xb in range(4):  # 2 token-chunks per stage tile
            t_ = xpool.tile([128, 2, C], BF16, tag="xs", name=f"xs{xb}")
            nc.gpsimd.dma_start(
                out=t_,
                in_=x_d[xb * 256:(xb + 1) * 256, :].rearrange(
                    "(k p) c -> p k c", p=128))
            xs.append(t_)

        wv_col = []
        wv_srcs = []
        for jvt in range(2):
            t_ = wpool.tile([128, CCH, 512], BF16, tag=f"wv{jvt}", name=f"wv{jvt}")
            wv_col.append(t_)
            wv_srcs.append(wq_d[:, 2 * C + jvt * 512: 2 * C + (jvt + 1) * 512])
        # q/k column blocks: qblk[b] = jt {2b,2b+1}, kblk[b] = jt {8+2b,9+2b}
        wqk_blk = {}

        def load_wqk_block(kind, b):
            t_ = wpool.tile([128, CCH, 256], BF16, tag=f"w{kind}{b}",
                            name=f"w{kind}{b}")
            base = b * 256 if kind == "q" else C + b * 256
            nc.gpsimd.dma_start(
                out=t_,
                in_=wq_d[:, base:base + 256].rearrange("(cc p) j -> p cc j", p=128))
            wqk_blk[(kind, b)] = t_

        def wq_ap(jt, cc):
            # lhsT slice for feature block jt (0..15), contraction chunk cc
            kind = "q" if jt < 8 else "k"
            jq = jt if jt < 8 else jt - 8
            t_ = wqk_blk[(kind, jq // 2)]
            u = jq % 2
            return t_[:, cc, u * 128:(u + 1) * 128]

        # wv0 then first two q/k blocks (pairs 0-3), then wv1, rest, wo
        nc.gpsimd.dma_start(
            out=wv_col[0], in_=wv_srcs[0].rearrange("(cc p) j -> p cc j", p=128))
        load_wqk_block("q", 0)
        load_wqk_block("k", 0)
        load_wqk_block("q", 1)
        load_wqk_block("k", 1)
        nc.gpsimd.dma_start(
            out=wv_col[1], in_=wv_srcs[1].rearrange("(cc p) j -> p cc j", p=128))
        load_wqk_block("q", 2)
        load_wqk_block("k", 2)
        load_wqk_block("q", 3)
        load_wqk_block("k", 3)
        # b_v on partitions for the bias fold, then W_out
        bv_pc = consts.tile([128, CCH], BF16, tag="bv_pc", name="bv_pc")
        nc.gpsimd.dma_start(
            out=bv_pc, in_=bq_d[:, 2 * C:3 * C].rearrange("x (cc p) -> p (x cc)", p=128))
        wo_col = wpool.tile([128, CCH, C], BF16, tag="wo", name="wo")
        nc.gpsimd.dma_start(
            out=wo_col, in_=wo_d.rearrange("(cc p) j -> p cc j", p=128))

        # ---- persistent activations ----
        xT = [apool.tile([128, T], BF16, tag=f"xT{cc}", name=f"xT{cc}")
              for cc in range(CCH)]
        qkT = [apool.tile([128, T], BF16, tag=f"qkT{jt}", name=f"qkT{jt}")
               for jt in range(16)]
        vp = [apool.tile([128, H * (D + 1)], BF16, tag=f"vp{t_}", name=f"vp{t_}")
              for t_ in range(TCH)]
        oT = [apool.tile([128, T], BF16, tag=f"oT{hc}", name=f"oT{hc}")
              for hc in range(CCH)]
        btilde = consts.tile([1, C], BF16, tag="btilde", name="btilde")

        n_dummy = [0]

        def dummy_mm():
            # PE heartbeat: keeps the HAM activity window non-idle so the
            # clock gate stays at 8/8. Result is never read.
            ps = PS.tile([128, 512], FP32, tag="aux", name=f"dmy{n_dummy[0]}", bufs=1)
            n_dummy[0] += 1
            nc.tensor.matmul(out=ps, lhsT=scratch[:, 0:128], rhs=scratch,
                             start=True, stop=True)

        # ---- opening: warmup + transposes (DMA-paced) ----
        for _ in range(6):
            dummy_mm()
        for ti in range(TCH):
            for cc in range(CCH):
                pt = PS.tile([128, 128], BF16, tag="po", name="tp", bufs=3)
                nc.tensor.transpose(
                    out=pt, in_=xs[ti // 2][:, ti % 2, cc * 128:(cc + 1) * 128],
                    identity=ident)
                nc.vector.tensor_copy(
                    out=xT[cc][:, ti * 128:(ti + 1) * 128], in_=pt)
            if ti % 2 == 1:
                dummy_mm()

        # ones columns of vp (denominator trick)
        for ti in range(TCH):
            vcol = vp[ti].rearrange("p (h d) -> p h d", h=H)
            nc.vector.memset(vcol[:, :, D:D + 1], 1.0)

        # ---- unit emitters ----
        def vproj_unit(ti, jvt, tag):
            ps = PS.tile([128, 512], FP32, tag=tag, name=f"psv{ti}_{jvt}",
                         bufs=1 if tag == "aux" else None)
            for cc in range(CCH):
                nc.tensor.matmul(
                    out=ps,
                    lhsT=xT[cc][:, ti * 128:(ti + 1) * 128],
                    rhs=wv_col[jvt][:, cc, :],
                    start=(cc == 0), stop=(cc == CCH - 1))
            vcol = vp[ti].rearrange("p (h d) -> p h d", h=H)
            nc.vector.tensor_copy(
                out=vcol[:, jvt * 8:(jvt + 1) * 8, 0:D],
                in_=ps.rearrange("p (h d) -> p h d", h=8))

        def qk_group(jt, half, tag):
            sl = slice(half * 512, (half + 1) * 512)
            ps = PS.tile([128, 512], FP32, tag=tag, name=f"psqk{jt}_{half}",
                         bufs=1 if tag == "aux" else None)
            has_bias = jt < 8  # q gets its bias; k-bias cancels in softmax
            for cc in range(CCH):
                nc.tensor.matmul(
                    out=ps, lhsT=wq_ap(jt, cc), rhs=xT[cc][:, sl],
                    start=(cc == 0),
                    stop=(cc == CCH - 1 and not has_bias))
            if has_bias:
                nc.tensor.matmul(
                    out=ps, lhsT=bqkv_sb[0:1, jt * 128:(jt + 1) * 128],
                    rhs=ones_row, start=False, stop=True)
            nc.vector.tensor_copy(out=qkT[jt][:, sl], in_=ps)

        def btilde_unit(half, tag):
            # b~ = b_v @ W_out + b_out   (one 512-wide half)
            sl = slice(half * 512, (half + 1) * 512)
            ps = PS.tile([128, 512], FP32, tag=tag, name=f"psbt{half}",
                         bufs=1 if tag == "aux" else None)
            for cc in range(CCH):
                nc.tensor.matmul(
                    out=ps[0:1, :], lhsT=bv_pc[:, cc:cc + 1],
                    rhs=wo_col[:, cc, sl],
                    start=(cc == 0), stop=(cc == CCH - 1))
            nc.vector.tensor_tensor(
                out=btilde[0:1, sl], in0=ps[0:1, :], in1=bout_sb[0:1, sl],
                op=mybir.AluOpType.add)

        # ---- opening compute: v proj (jvt0) + first qk pair ----
        for ti in range(TCH):
            vproj_unit(ti, 0, "ps")
            if ti % 3 == 2:
                dummy_mm()
        for g in ((0, 0), (8, 0), (0, 1), (8, 1)):
            qk_group(*g, "ps")

        # ---- filler queue for the attention phase ----
        filler = []
        for p in range(1, 8):
            for half in range(2):
                filler.append(("qk", p, half))       # q side
                filler.append(("qk", 8 + p, half))   # k side
            if p == 3:
                for ti in range(TCH):
                    filler.append(("vp", ti, 1))
            if p == 5:
                filler.append(("bt", 0, None))
                filler.append(("bt", 1, None))
        fill_pos = [0]

        def emit_filler(tag):
            if fill_pos[0] >= len(filler):
                dummy_mm()
                return
            kind, a, b = filler[fill_pos[0]]
            fill_pos[0] += 1
            if kind == "qk":
                qk_group(a, b, tag)
            elif kind == "vp":
                vproj_unit(a, b, tag)
            else:
                btilde_unit(a, tag)

        def qk_pairs_done(hp):
            # number of fully emitted qk pairs (pair p needs 4 groups)
            done = 4 + fill_pos[0] - sum(
                1 for i in range(fill_pos[0]) if filler[i][0] != "qk")
            return done >= 4 * (hp + 1)

        # ---- attention ----
        for hp in range(8):
            # correctness guard: pair hp's projections must be emitted first
            while not qk_pairs_done(hp):
                emit_filler("aux")
            h0, h1 = 2 * hp, 2 * hp + 1
            qk_q, qk_k = qkT[hp], qkT[8 + hp]
            for it in range(2):
                njc = 4 * (it + 1)
                po2 = [PS.tile([65, 512], FP32, tag="po", name=f"po{hx}", bufs=3)
                       for hx in range(2)]
                for jc in range(njc):
                    s0 = max(0, jc * 128 - it * 512)
                    ps = PS.tile([128, 2, 512], FP32, tag="ps", name="pss")
                    for hx in range(2):
                        prow = slice(hx * 64, hx * 64 + 64)
                        nc.tensor.matmul(
                            out=ps[:, hx, s0:512],
                            lhsT=qk_k[prow, jc * 128:(jc + 1) * 128],
                            rhs=qk_q[prow, it * 512 + s0:(it + 1) * 512],
                            start=True, stop=True)
                    pT = ppool.tile([128, 2, 512], BF16, tag="pT", name="pT")
                    nc.scalar.activation(
                        out=pT[:, :, s0:512], in_=ps[:, :, s0:512],
                        func=mybir.ActivationFunctionType.Exp, scale=0.125)
                    if jc >= it * 4:  # diagonal block: zero the upper triangle
                        nc.vector.tensor_tensor(
                            out=pT[:, :, s0:s0 + 128],
                            in0=pT[:, :, s0:s0 + 128],
                            in1=mask01[:, None, :].to_broadcast([128, 2, 128]),
                            op=mybir.AluOpType.mult)
                    if jc % 2 == 1:
                        emit_filler("aux")
                    for hx, h in enumerate((h0, h1)):
                        hsl = slice(h * (D + 1), h * (D + 1) + D + 1)
                        nc.tensor.matmul(
                            out=po2[hx][0:65, s0:512],
                            lhsT=vp[jc][:, hsl],
                            rhs=pT[:, hx, s0:512],
                            start=(jc == 0), stop=(jc == njc - 1),
                            skip_group_check=True)
                # normalize: row 64 of po = l = sum_j p
                for hx in range(2):
                    po = po2[hx]
                    l_sb = lpool.tile([1, 512], BF16, tag="l", name="l")
                    nc.scalar.copy(out=l_sb, in_=po[64:65, :])
                    plb = PS.tile([64, 512], FP32, tag="ps", name="plb")
                    nc.tensor.matmul(out=plb, lhsT=ones64, rhs=l_sb,
                                     start=True, stop=True)
                    rb = rbpool.tile([64, 512], FP32, tag="rb", name="rb")
                    nc.vector.reciprocal_approx_fast(out=rb, in_=plb)
                    prow = slice(hx * 64, hx * 64 + 64)
                    nc.vector.tensor_tensor(
                        out=oT[hp][prow, it * 512:(it + 1) * 512],
                        in0=po[0:64, :], in1=rb, op=mybir.AluOpType.mult)

        # drain any leftover filler (normally empty)
        while fill_pos[0] < len(filler):
            emit_filler("aux")

        # ---- output projection ----
        for ti in range(TCH):
            ot = outs.tile([128, C], FP32, tag="ot", name="ot")
            for half in range(2):
                sl = slice(half * 512, (half + 1) * 512)
                ps = PS.tile([128, 512], FP32, tag="ps", name="pso")
                for hc in range(CCH):
                    nc.tensor.matmul(
                        out=ps,
                        lhsT=oT[hc][:, ti * 128:(ti + 1) * 128],
                        rhs=wo_col[:, hc, sl],
                        start=(hc == 0), stop=False)
                nc.tensor.matmul(
                    out=ps, lhsT=ones_row[0:1, 0:128], rhs=btilde[0:1, sl],
                    start=False, stop=True)
                nc.vector.tensor_copy(out=ot[:, sl], in_=ps)
            nc.sync.dma_start(out=out_d[ti * 128:(ti + 1) * 128, :], in_=ot)

    nc.compile()
    nc.finalize()
    return nc


_CACHE = {}


def _host_consts():
    ident = np.eye(128, dtype=ml_dtypes.bfloat16)
    ii = np.arange(128)
    mask01 = (ii[None, :] >= ii[:, None]).astype(ml_dtypes.bfloat16)
    return ident, mask01


def kernel(x, W_qkv, b_qkv, W_out, b_out):
    if "nc" not in _CACHE:
        _CACHE["nc"] = build_nc()
    nc = _CACHE["nc"]
    x = np.ascontiguousarray(np.asarray(x, dtype=np.float32))
    ident, mask01 = _host_consts()
    in_maps = [
        {
            "x_b": x[i],
            "w_qkv": np.ascontiguousarray(np.asarray(W_qkv, np.float32)),
            "b_qkv": np.ascontiguousarray(np.asarray(b_qkv, np.float32).reshape(1, -1)),
            "w_out": np.ascontiguousarray(np.asarray(W_out, np.float32)),
            "b_out": np.ascontiguousarray(np.asarray(b_out, np.float32).reshape(1, -1)),
            "ident": ident,
            "mask01": mask01,
        }
        for i in range(N_CORES)
    ]
    res = bass_utils.run_bass_kernel_spmd(nc, in_maps, core_ids=list(range(N_CORES)))
    return np.stack([r["out_b"] for r in res.results]).astype(np.float32)
